# revision 25
# baseline (speedup 1.0000x reference)
"""NT-Xent loss kernel for Trainium2, 8-core SPMD.

Strategy (row-parallel, hint-compliant):
  - Host: z = concat(z1, z2) [8192, 256]; pass z^T (bf16) to each core with
    columns ROTATED so core c's own 1024 rows sit at columns 0..1023 and the
    positive-pair partner rows sit at columns 4096..5119 (fixed positions ->
    one SPMD program, no per-core addressing).  A small f32 copy of the own +
    partner columns feeds the fp32-accurate positives path.
  - Device (per core): normalize all 8192 columns of z^T on-chip, compute the
    [1024 x 8192] block of the similarity matrix with bf16 matmuls, mask the
    self-sim diagonal by accumulating -16*I into the diagonal 128-block (exp
    underflows to exactly 0), and reduce exp row-sums with the ACT engine's
    fused accumulate output.  Each core emits log(denom)-pos for its rows.
  - Host: mean over the 8 x [128, 8] partial outputs -> scalar loss.

Emission is pipelined: norm-chain per 2048-col j-tile, then each j-tile's
main-loop block right after its normalize, so PE/ACT start early while the
DVE works on later tiles.  All ACT Sqrt ops complete before the first Exp
(different ACT table sets; switches cost ~2.7us).
"""

import numpy as np
import ml_dtypes

import concourse.bass as bass
import concourse.tile as tile
from concourse import bacc, mybir
from concourse import bass_utils

F32 = mybir.dt.float32
BF16 = mybir.dt.bfloat16
AF = mybir.ActivationFunctionType
AX = mybir.AxisListType

P = 128          # partitions
D = 256          # feature dim
NR = 8192        # total rows (2N)
KT = D // P      # k-tiles (2)
NCORE = 8
RPC = NR // NCORE  # rows per core (1024)
MT = RPC // P      # m-tiles per core (8)
JTILE = 2048
NJ = NR // JTILE   # 4 j-tiles
JW = 1024
TEMP = 0.07
DIAG_C = 16.0      # diagonal shift: (1 - 16)/temp = -214 -> exp == 0


def build_kernel():
    nc = bacc.Bacc(
        "TRN2",
        target_bir_lowering=False,
        debug=False,
        enable_asserts=True,
        num_devices=NCORE,
    )
    zaT = nc.dram_tensor("zaT", [D, NR], BF16, kind="ExternalInput").ap()
    zpos = nc.dram_tensor("zpos", [D, 2 * JW], F32, kind="ExternalInput").ap()
    eye = nc.dram_tensor("eye", [P, P], BF16, kind="ExternalInput").ap()
    negeye = nc.dram_tensor("negeye", [P, P], BF16, kind="ExternalInput").ap()
    loss_out = nc.dram_tensor("loss_out", [P, MT], F32, kind="ExternalOutput").ap()

    with tile.TileContext(nc) as tc:
        with (
            tc.tile_pool(name="persist", bufs=1) as persist,
            tc.tile_pool(name="scratch", bufs=3) as scratch,
            tc.tile_pool(name="load", bufs=1) as loadp,
            tc.tile_pool(name="presm", bufs=1) as presm,
            tc.tile_pool(name="sqp", bufs=4) as sqp,
        ):
            # --- constants ---
            ones_b = persist.tile([P, P], BF16, tag="ones_b")
            nc.vector.memset(ones_b[:], 1.0)
            junk = persist.tile([P, 1], F32, tag="junk")
            nc.vector.memset(junk[:], 0.0)
            ones_f = persist.tile([P, 1], F32, tag="ones_f")
            nc.vector.memset(ones_f[:], 1.0)
            eye_sb = persist.tile([P, P], BF16, tag="eye_sb")
            nc.sync.dma_start(out=eye_sb[:], in_=eye)
            negeye_sb = persist.tile([P, P], BF16, tag="negeye_sb")
            nc.sync.dma_start(out=negeye_sb[:], in_=negeye)


            # --- load z^T (bf16), j-tile-major so early tiles complete first ---
            zin = {}
            for j in range(NJ):
                for k in range(KT):
                    t = loadp.tile([P, JTILE], BF16, tag=f"zin_{k}_{j}")
                    nc.sync.dma_start(
                        out=t[:],
                        in_=zaT[k * P:(k + 1) * P, j * JTILE:(j + 1) * JTILE],
                    )
                    zin[k, j] = t
            # f32 own+partner columns for the positives path
            zposin = {}
            for k in range(KT):
                t = loadp.tile([P, 2 * JW], F32, tag=f"zpos_{k}")
                nc.sync.dma_start(out=t[:], in_=zpos[k * P:(k + 1) * P, :])
                zposin[k] = t

            # --- norm chain per j-tile: sq -> colsum matmul -> sqrt -> recip ---
            sq = {}
            rstdf = {}   # f32 rstd per j-tile [P, JTILE]
            rstd16 = {}  # bf16 copy for the 2x normalize mul
            znb = {}     # normalized bf16 matmul operands
            with tc.tile_pool(name="pps", bufs=2, space="PSUM") as pps:
                # all sq muls + colsum matmuls + sqrts FIRST: ACT is strict
                # FIFO, so every Sqrt must clear before the first Exp can
                # issue; gate them only on the cheap DVE sq muls.
                for j in range(NJ):
                    for k in range(KT):
                        t = sqp.tile([P, JTILE], BF16, tag="sqt")
                        nc.vector.tensor_mul(t[:], zin[k, j][:], zin[k, j][:])
                        sq[k, j] = t
                    ps = pps.tile([P, JTILE], F32, tag="nps")
                    for c2 in range(4):  # 512-wide (PSUM bank limit)
                        lo = c2 * 512
                        for k in range(KT):
                            nc.tensor.matmul(
                                ps[:, lo:lo + 512],
                                lhsT=ones_b[:],
                                rhs=sq[k, j][:, lo:lo + 512],
                                start=(k == 0),
                                stop=(k == KT - 1),
                            )
                    rt = presm.tile([P, JTILE], F32, tag=f"rstdf_{j}")
                    nc.scalar.activation(rt[:], ps[:], AF.Sqrt)
                    rstdf[j] = rt
                # dummy exp right after the sqrts in ACT's FIFO: triggers the
                # exp-set table load (~2.7us) while the DVE normalize chain
                # runs, instead of serializing before the first real exp
                nc.scalar.activation(junk[:], junk[:], AF.Exp)
                # then per j: recip + bf16 copy + normalize (DVE only)
                for j in range(NJ):
                    nc.vector.reciprocal(rstdf[j][:], rstdf[j][:])
                    r16 = presm.tile([P, JTILE], BF16, tag=f"rstd16_{j}")
                    nc.vector.tensor_copy(r16[:], rstdf[j][:])
                    rstd16[j] = r16
                    for k in range(KT):
                        t = persist.tile([P, JTILE], BF16, tag=f"znb_{k}_{j}")
                        nc.vector.tensor_mul(t[:], zin[k, j][:], r16[:])
                        znb[k, j] = t

            # --- main loop per column super-tile ---
            dparts = persist.tile([P, MT, NJ], F32, tag="dparts")
            pos_sb = persist.tile([P, MT], F32, tag="pos_sb")

            with tc.tile_pool(name="mps", bufs=2, space="PSUM") as mps:
                for ns in range(NJ):
                    # sim block rows for this column super-tile
                    for m in range(MT):
                        ps = mps.tile([P, JTILE], F32, tag="simps")
                        for c5 in range(4):  # 512-wide matmuls
                            lo = c5 * 512
                            is_diag_chunk = (ns == 0) and (lo <= m * P < lo + 512)
                            for k in range(KT):
                                nc.tensor.matmul(
                                    ps[:, lo:lo + 512],
                                    lhsT=znb[k, 0][:, m * P:(m + 1) * P],
                                    rhs=znb[k, ns][:, lo:lo + 512],
                                    start=(k == 0),
                                    stop=(k == KT - 1),
                                )
                            if is_diag_chunk:
                                nc.tensor.matmul(
                                    ps[:, m * P:(m + 1) * P],
                                    lhsT=eye_sb[:],
                                    rhs=negeye_sb[:],
                                    start=False,
                                    stop=False,
                                    skip_group_check=True,
                                )
                        esc = scratch.tile([P, JTILE], BF16, tag="esc")
                        nc.scalar.activation(
                            esc[:],
                            ps[:],
                            AF.Exp,
                            scale=1.0 / TEMP,
                            accum_out=dparts[:, m, ns:ns + 1],
                        )

                    if ns == 2:
                        # positives (fp32): own rows (zpos cols 0..1023,
                        # rstd chunk j0 lower half) x partner rows (zpos cols
                        # 1024..2047, rstd j2 lower half)
                        prod = {}
                        for k in range(KT):
                            zr = presm.tile([P, JW], F32, tag=f"zrnf_{k}")
                            nc.vector.tensor_mul(
                                zr[:], zposin[k][:, 0:JW], rstdf[0][:, 0:JW]
                            )
                            zp = presm.tile([P, JW], F32, tag=f"zpnf_{k}")
                            nc.vector.tensor_mul(
                                zp[:], zposin[k][:, JW:2 * JW], rstdf[2][:, 0:JW]
                            )
                            pr = presm.tile([P, JW], F32, tag=f"prod_{k}")
                            nc.vector.tensor_mul(pr[:], zr[:], zp[:])
                            prod[k] = pr
                        # borrows one of the two sim-psum slots briefly
                        pos_ps = mps.tile([P, MT], F32, tag="simps")
                        for m in range(MT):
                            for k in range(KT):
                                nc.tensor.matmul(
                                    pos_ps[:, m:m + 1],
                                    lhsT=prod[k][:, m * P:(m + 1) * P],
                                    rhs=ones_f[:],
                                    start=(k == 0),
                                    stop=(k == KT - 1),
                                )
                        nc.vector.tensor_copy(pos_sb[:], pos_ps[:])

                # --- tail: loss = log(sum exp) - pos/temp ---
                dsum = persist.tile([P, MT], F32, tag="dsum")
                nc.vector.reduce_sum(out=dsum[:], in_=dparts[:], axis=AX.X)
                lvec = persist.tile([P, MT], F32, tag="lvec")
                nc.scalar.activation(lvec[:], dsum[:], AF.Ln)
                post = persist.tile([P, MT], F32, tag="post")
                nc.vector.tensor_scalar_mul(post[:], pos_sb[:], 1.0 / TEMP)
                loss_sb = persist.tile([P, MT], F32, tag="loss_sb")
                nc.vector.tensor_sub(loss_sb[:], lvec[:], post[:])
                nc.sync.dma_start(out=loss_out, in_=loss_sb[:])

    nc.compile()
    return nc


_NC_CACHE = None


def _get_nc():
    global _NC_CACHE
    if _NC_CACHE is None:
        _NC_CACHE = build_kernel()
    return _NC_CACHE


def make_in_maps(z1: np.ndarray, z2: np.ndarray):
    z = np.concatenate([np.asarray(z1), np.asarray(z2)], axis=0).astype(np.float32)
    zaT = np.ascontiguousarray(z.T)  # [256, 8192] f32
    zaT16 = zaT.astype(ml_dtypes.bfloat16)
    eye_np = np.eye(P, dtype=ml_dtypes.bfloat16)
    negeye_np = (-DIAG_C * np.eye(P)).astype(ml_dtypes.bfloat16)
    in_maps = []
    for c in range(NCORE):
        s = c * RPC
        rot16 = np.concatenate([zaT16[:, s:], zaT16[:, :s]], axis=1)
        own = zaT[:, s:s + RPC]
        pstart = (s + NR // 2) % NR
        partner = zaT[:, pstart:pstart + RPC]
        in_maps.append(
            {
                "zaT": np.ascontiguousarray(rot16),
                "zpos": np.ascontiguousarray(
                    np.concatenate([own, partner], axis=1)
                ),
                "eye": eye_np,
                "negeye": negeye_np,
            }
        )
    return in_maps


def kernel(z1: np.ndarray, z2: np.ndarray) -> np.ndarray:
    nc = _get_nc()
    in_maps = make_in_maps(z1, z2)
    res = bass_utils.run_bass_kernel_spmd(nc, in_maps, core_ids=list(range(NCORE)))
    parts = [res.results[c]["loss_out"] for c in range(NCORE)]
    # each [p, m] entry is loss for one row; global mean over all 2N rows
    total = np.stack(parts).astype(np.float64)
    return np.float32(total.mean())
